# revision 1
# baseline (speedup 1.0000x reference)
"""Trainium2 Bass kernel for nn_HODE_MDP (hypergraph ODE message passing).

Math (T_UP = T_GEO = T_P2P = 1.0, ALPHA = 0.8):
    pe  = poi_emb_weight[:-1]                      # [P, D]
    x/s/g = pe * sigmoid(pe @ W_t + b_t)           # col / seq / geo gates
    hg_pois    = x + HG_pu @ (HG_up @ x)
    geo_pois   = g + 0.4 * (poi_geo_graph @ g)
    trans_pois = s + HG_poi_src @ (HG_poi_tar @ s)
    hg_users   = (HG_up @ hg_pois)[user_idx]
    geo_users  = (HG_up @ geo_pois)[user_idx]
    out = concat([hg_pois, geo_pois, trans_pois, hg_users, geo_users])

Distribution (8 NeuronCores): shard the row dim of every big matrix
(P rows for HG_pu / HG_poi_src / poi_geo_graph, U rows for HG_up, E rows
for HG_poi_tar).  Each core computes its row-block of each product with
the full activation.  Matrix blocks are shipped pre-transposed
([contract_dim, out_block]) so the PE consumes them as the moving
operand with N=512; the activation k-tile [128, 128] is the stationary
operand.  All products run in bf16 (f32 PSUM accum); the ODE deltas are
~1e-5 of the output scale so bf16 matmul error is negligible (measured
l2 rel err ~1.5e-5 end to end vs the f32 reference).

The gate activations are REPLICATED (every core computes all of x/s/g
from the full 4MB embedding) — on this stack an AllGather costs
~25-70us of latency, so gathering [P, D] gate tensors serialized the
whole kernel.  Only the four small downstream tensors (y_up, y_tar,
geo_pois, hg_pois) are all-gathered, each launched as early as possible
so its latency hides under the matrix streams.

Outputs come back transposed [D, block]; the host re-transposes,
concatenates, and applies the user_idx gather.
"""

import sys

if "/opt/trn_rl_repo" not in sys.path:
    sys.path.insert(0, "/opt/trn_rl_repo")

import numpy as np
import ml_dtypes

import concourse.bass as bass  # noqa: F401
import concourse.bacc as bacc
import concourse.mybir as mybir
import concourse.tile as tile
from concourse.bass_utils import run_bass_kernel_spmd

F32 = mybir.dt.float32
BF16 = mybir.dt.bfloat16
SIG = mybir.ActivationFunctionType.Sigmoid
MULT = mybir.AluOpType.mult
ADD = mybir.AluOpType.add
BYPASS = mybir.AluOpType.bypass

NCORES = 8
P, U, E, D = 8192, 4096, 4096, 128
PP, UU, EE = P // NCORES, U // NCORES, E // NCORES  # 1024, 512, 512
KP, KU = P // 128, U // 128                         # 64, 32 k-tiles
RG = [list(range(NCORES))]
GEO_SCALE = 0.4  # ALPHA / 2 * T_GEO

_CACHE: dict = {}


def _build_nc():
    nc = bacc.Bacc(
        "TRN2",
        target_bir_lowering=False,
        debug=False,
        enable_asserts=False,
        num_devices=NCORES,
    )

    # ---- per-core DRAM I/O ----------------------------------------------
    peT = nc.dram_tensor("peT", [D, PP], F32, kind="ExternalInput").ap()
    peTf = nc.dram_tensor("peTf", [D, P], F32, kind="ExternalInput").ap()
    w3 = nc.dram_tensor("w3", [3, D, D], F32, kind="ExternalInput").ap()
    bT3 = nc.dram_tensor("bT3", [D, 3], F32, kind="ExternalInput").ap()
    ident = nc.dram_tensor("ident", [D, D], F32, kind="ExternalInput").ap()
    UpT = nc.dram_tensor("UpT", [P, UU], BF16, kind="ExternalInput").ap()
    PuT = nc.dram_tensor("PuT", [U, PP], BF16, kind="ExternalInput").ap()
    TarT = nc.dram_tensor("TarT", [P, EE], BF16, kind="ExternalInput").ap()
    SrcT = nc.dram_tensor("SrcT", [E, PP], BF16, kind="ExternalInput").ap()
    GeoT = nc.dram_tensor("GeoT", [P, PP], BF16, kind="ExternalInput").ap()

    poisT_o = nc.dram_tensor("poisT_o", [3, D, PP], F32, kind="ExternalOutput").ap()
    usersT_o = nc.dram_tensor("usersT_o", [D, 2 * UU], F32, kind="ExternalOutput").ap()

    with tile.TileContext(nc) as tc:
        with (
            tc.tile_pool(name="const", bufs=1) as constp,
            tc.tile_pool(name="big", bufs=3) as bigp,
            tc.tile_pool(name="rhs", bufs=6) as rhsp,
            tc.tile_pool(name="stage", bufs=2) as stagep,
            tc.tile_pool(name="outp", bufs=2) as outp,
            tc.tile_pool(name="psacc", bufs=2, space="PSUM") as psacc,
            tc.tile_pool(name="pstr", bufs=2, space="PSUM") as pstr,
            tc.tile_pool(name="dram", bufs=1, space="DRAM") as dramp,
        ):
            # ---- internal DRAM collective buffers (SBUF-mirror layout) --
            # [128, cols]: contiguous per-partition lines for every bounce
            # DMA; AllGather stacks rank blocks along axis 0.
            cc_yu_in = dramp.tile([128, UU], BF16, name="cc_yu_in")
            cc_yu_out = dramp.tile(
                [NCORES * 128, UU], BF16, addr_space="Shared", name="cc_yu_out"
            )
            cc_yt_in = dramp.tile([128, EE], BF16, name="cc_yt_in")
            cc_yt_out = dramp.tile(
                [NCORES * 128, EE], BF16, addr_space="Shared", name="cc_yt_out"
            )
            cc_geo_in = dramp.tile([128, PP], BF16, name="cc_geo_in")
            cc_geo_out = dramp.tile(
                [NCORES * 128, PP], BF16, addr_space="Shared", name="cc_geo_out"
            )
            cc_hg_in = dramp.tile([128, PP], BF16, name="cc_hg_in")
            cc_hg_out = dramp.tile(
                [NCORES * 128, PP], BF16, addr_space="Shared", name="cc_hg_out"
            )

            def allgather(cc_in, cc_out):
                nc.gpsimd.collective_compute(
                    "AllGather",
                    BYPASS,
                    replica_groups=RG,
                    ins=[cc_in[:].opt()],
                    outs=[cc_out[:].opt()],
                )

            # ---- constants (gpsimd: sync/scalar queues kept pure stream) -
            sb_ident = constp.tile([D, D], F32, name="sb_ident")
            nc.gpsimd.dma_start(sb_ident[:], ident)
            sb_w = constp.tile([D, 3, D], F32, name="sb_w")
            nc.gpsimd.dma_start(sb_w[:], w3.rearrange("t k m -> k t m"))
            sb_wb = constp.tile([D, 3, D], BF16, name="sb_wb")
            nc.vector.tensor_copy(sb_wb[:], sb_w[:])
            sb_bT = constp.tile([D, 3], F32, name="sb_bT")
            nc.gpsimd.dma_start(sb_bT[:], bT3)
            sb_peT = constp.tile([D, PP], F32, name="sb_peT")
            nc.gpsimd.dma_start(sb_peT[:], peT)
            sb_peTf = constp.tile([D, P], F32, name="sb_peTf")
            nc.gpsimd.dma_start(sb_peTf[:], peTf)

            # own-block gate results, transposed [D, PP] f32, kept resident
            sb_gateT = [
                constp.tile([D, PP], F32, name=f"sb_gateT{t}") for t in range(3)
            ]

            def gate_own(t):
                """Own row-block gate (f32): sb_gateT[t] = peT*sig(peT@W+b)."""
                psg = psacc.tile([D, PP], F32, tag="acc")
                for h in range(2):
                    cols = slice(512 * h, 512 * (h + 1))
                    nc.tensor.matmul(
                        psg[:, cols], sb_w[:, t, :], sb_peT[:, cols],
                        start=True, stop=True,
                    )
                sb_sig = stagep.tile([D, PP], F32, tag="sigo")
                for h in range(2):
                    cols = slice(512 * h, 512 * (h + 1))
                    nc.scalar.activation(
                        sb_sig[:, cols], psg[:, cols], SIG, bias=sb_bT[:, t : t + 1]
                    )
                nc.vector.tensor_mul(sb_gateT[t], sb_peT[:], sb_sig[:])

            def gate_full(t, name):
                """Full replicated gate -> natural bf16 k-tiles [128, (k d)].

                z chunks use bf16 matmuls (error lands ~1e-5 of output);
                sigmoid + pe-mul stay f32; PE-transpose into natural tiles.
                """
                g_nat = bigp.tile([128, P], BF16, tag="big", name=name)
                for h in range(P // 512):
                    cols = slice(512 * h, 512 * (h + 1))
                    peb = stagep.tile([D, 512], BF16, tag="peb")
                    nc.vector.tensor_copy(peb[:], sb_peTf[:, cols])
                    psz = psacc.tile([D, 512], F32, tag="acc")
                    nc.tensor.matmul(
                        psz[:], sb_wb[:, t, :], peb[:], start=True, stop=True
                    )
                    sigc = stagep.tile([D, 512], F32, tag="sigc")
                    nc.scalar.activation(
                        sigc[:], psz[:], SIG, bias=sb_bT[:, t : t + 1]
                    )
                    xtc = stagep.tile([D, 512], F32, tag="xtc")
                    nc.vector.tensor_mul(xtc[:], sb_peTf[:, cols], sigc[:])
                    for m in range(4):
                        pst = pstr.tile([128, 128], F32, tag="tr")
                        nc.tensor.transpose(
                            pst[:], xtc[:, m * 128 : (m + 1) * 128], sb_ident[:]
                        )
                        nc.vector.tensor_copy(
                            g_nat[:, (4 * h + m) * 128 : (4 * h + m + 1) * 128],
                            pst[:],
                        )
                return g_nat

            def nat_store(srcT, cc_dst, n_m):
                """PE-transpose [D, n_m*128] srcT (f32) into natural k-tiles
                and DMA (cast to bf16) into cc_dst ([128, n_m*128] DRAM)."""
                nat = stagep.tile([128, n_m * 128], BF16, tag="nat")
                for m in range(n_m):
                    pst = pstr.tile([128, 128], F32, tag="tr")
                    nc.tensor.transpose(
                        pst[:], srcT[:, m * 128 : (m + 1) * 128], sb_ident[:]
                    )
                    nc.vector.tensor_copy(nat[:, m * 128 : (m + 1) * 128], pst[:])
                nc.gpsimd.dma_start(cc_dst, nat[:])

            def load_full(cc_out, col0, cols, name):
                """Gather rank blocks of an all-gathered SBUF-mirror tensor
                into one SBUF tile of natural k-tiles."""
                full = bigp.tile([128, NCORES * cols], BF16, tag="big", name=name)
                for r in range(NCORES):
                    nc.gpsimd.dma_start(
                        full[:, r * cols : (r + 1) * cols],
                        cc_out[r * 128 : (r + 1) * 128, col0 : col0 + cols],
                    )
                return full

            def stream_product(lhs_full, matT, n_k, n_out, psum_tiles, extra=None):
                """psum[d, n_out] += sum_k lhs_full_tile_k.T @ matT[k-tile, :].

                matT: DRAM [n_k*128, n_out] bf16, streamed in ~1MB chunks on
                the sync/scalar HWDGE queues (no dependencies -> pure
                prefetch).  extra: (lhs_full2, psum2) fuses a second product
                sharing the same rhs stream.
                """
                n512 = n_out // 512
                ck = max(1, (1 << 20) // (n_out * 2 * 128))  # k-tiles / ~1MB
                n_chunks = (n_k + ck - 1) // ck
                for c in range(n_chunks):
                    k0 = c * ck
                    kn = min(ck, n_k - k0)
                    chunk = rhsp.tile([128, ck, n_out], BF16, tag="rhs")
                    eng = nc.sync if c % 2 == 0 else nc.scalar
                    eng.dma_start(
                        chunk[:, :kn, :],
                        matT[k0 * 128 : (k0 + kn) * 128, :].rearrange(
                            "(a p) n -> p a n", p=128
                        ),
                    )
                    for kk in range(kn):
                        k = k0 + kk
                        lhs_tile = lhs_full[:, k * 128 : (k + 1) * 128]
                        for n in range(n512):
                            cols = slice(512 * n, 512 * (n + 1))
                            nc.tensor.matmul(
                                psum_tiles[n][:, :],
                                lhs_tile,
                                chunk[:, kk, cols],
                                start=(k == 0),
                                stop=(k == n_k - 1),
                            )
                        if extra is not None:
                            lhs2, ps2 = extra
                            nc.tensor.matmul(
                                ps2[:, :],
                                lhs2[:, k * 128 : (k + 1) * 128],
                                chunk[:, kk, 0:512],
                                start=(k == 0),
                                stop=(k == n_k - 1),
                            )

            # ---- phase A0: x gate (full + own block) --------------------
            x_nat = gate_full(0, "x_nat")
            gate_own(0)

            # ---- phase B1: y_up = HG_up @ x  (shard over U rows) --------
            ps_yu = psacc.tile([D, 512], F32, tag="acc")
            stream_product(x_nat, UpT, KP, UU, [ps_yu])
            yuT = stagep.tile([D, UU], F32, tag="ysb")
            nc.vector.tensor_copy(yuT[:], ps_yu[:])
            nat_store(yuT, cc_yu_in[:, :], UU // 128)
            allgather(cc_yu_in, cc_yu_out)

            # ---- phase A1: s gate (overlaps AG_yu) ----------------------
            s_nat = gate_full(1, "s_nat")
            gate_own(1)

            # ---- phase B2: y_tar = HG_poi_tar @ s  (shard over E rows) --
            ps_yt = psacc.tile([D, 512], F32, tag="acc")
            stream_product(s_nat, TarT, KP, EE, [ps_yt])
            ytT = stagep.tile([D, EE], F32, tag="ysb")
            nc.vector.tensor_copy(ytT[:], ps_yt[:])
            nat_store(ytT, cc_yt_in[:, :], EE // 128)
            allgather(cc_yt_in, cc_yt_out)

            # ---- phase A2: g gate -----------------------------------------
            g_nat = gate_full(2, "g_nat")
            gate_own(2)

            # ---- phase B3: geo_pois = g + 0.4 * Geo @ g  (P-row shard) --
            ps_geo = psacc.tile([D, PP], F32, tag="acc")
            stream_product(
                g_nat, GeoT, KP, PP, [ps_geo[:, 0:512], ps_geo[:, 512:1024]]
            )
            geo_poisT = outp.tile([D, PP], F32, tag="out", name="geo_poisT")
            nc.vector.scalar_tensor_tensor(
                geo_poisT[:], ps_geo[:], GEO_SCALE, sb_gateT[2][:], MULT, ADD
            )
            nc.gpsimd.dma_start(poisT_o[1], geo_poisT[:])
            nat_store(geo_poisT, cc_geo_in[:, :], PP // 128)
            allgather(cc_geo_in, cc_geo_out)

            # ---- gathered y_up ------------------------------------------
            yup_full = load_full(cc_yu_out, 0, UU, "yup_full")

            # ---- phase C1: hg_pois = x + HG_pu @ y_up  (P-row shard) ----
            ps_hg = psacc.tile([D, PP], F32, tag="acc")
            stream_product(
                yup_full, PuT, KU, PP, [ps_hg[:, 0:512], ps_hg[:, 512:1024]]
            )
            hg_poisT = outp.tile([D, PP], F32, tag="out", name="hg_poisT")
            nc.vector.tensor_add(hg_poisT[:], ps_hg[:], sb_gateT[0][:])
            nc.gpsimd.dma_start(poisT_o[0], hg_poisT[:])
            nat_store(hg_poisT, cc_hg_in[:, :], PP // 128)
            allgather(cc_hg_in, cc_hg_out)

            # ---- gathered y_tar -----------------------------------------
            ytar_full = load_full(cc_yt_out, 0, EE, "ytar_full")

            # ---- phase C2: trans_pois = s + Src @ y_tar  (P-row shard) --
            ps_tr = psacc.tile([D, PP], F32, tag="acc")
            stream_product(
                ytar_full, SrcT, KU, PP, [ps_tr[:, 0:512], ps_tr[:, 512:1024]]
            )
            trans_poisT = outp.tile([D, PP], F32, tag="out", name="trans_poisT")
            nc.vector.tensor_add(trans_poisT[:], ps_tr[:], sb_gateT[1][:])
            nc.gpsimd.dma_start(poisT_o[2], trans_poisT[:])

            # ---- gathered pois ------------------------------------------
            geo_full = load_full(cc_geo_out, 0, PP, "geo_full")
            hg_full = load_full(cc_hg_out, 0, PP, "hg_full")

            # ---- phase D: user embeddings (shard over U rows) -----------
            ps_hu = psacc.tile([D, 512], F32, tag="acc")
            ps_gu = psacc.tile([D, 512], F32, tag="acc")
            stream_product(hg_full, UpT, KP, UU, [ps_hu], extra=(geo_full, ps_gu))
            users_sb = outp.tile([D, 2 * UU], F32, tag="out", name="users_sb")
            nc.vector.tensor_copy(users_sb[:, 0:UU], ps_hu[:])
            nc.vector.tensor_copy(users_sb[:, UU : 2 * UU], ps_gu[:])
            nc.gpsimd.dma_start(usersT_o, users_sb[:])

    nc.compile()
    return nc


def _get_nc():
    if "nc" not in _CACHE:
        _CACHE["nc"] = _build_nc()
    return _CACHE["nc"]


def _shard_inputs(inputs):
    f32 = np.float32
    bf16 = ml_dtypes.bfloat16
    pe = np.asarray(inputs["poi_emb_weight"], f32)[:P]
    peTf = np.ascontiguousarray(pe.T)
    w3 = np.stack(
        [
            np.asarray(inputs["w_gate_col"], f32),
            np.asarray(inputs["w_gate_seq"], f32),
            np.asarray(inputs["w_gate_geo"], f32),
        ]
    )
    bT3 = np.stack(
        [
            np.asarray(inputs["b_gate_col"], f32)[0],
            np.asarray(inputs["b_gate_seq"], f32)[0],
            np.asarray(inputs["b_gate_geo"], f32)[0],
        ],
        axis=1,
    )
    eye = np.eye(D, dtype=f32)
    Up = np.asarray(inputs["HG_up"], f32)
    Pu = np.asarray(inputs["HG_pu"], f32)
    Tar = np.asarray(inputs["HG_poi_tar"], f32)
    Src = np.asarray(inputs["HG_poi_src"], f32)
    Geo = np.asarray(inputs["poi_geo_graph"], f32)

    in_maps = []
    for i in range(NCORES):
        rp = slice(PP * i, PP * (i + 1))
        ru = slice(UU * i, UU * (i + 1))
        re_ = slice(EE * i, EE * (i + 1))
        in_maps.append(
            {
                "peT": peTf[:, rp].copy(),
                "peTf": peTf,
                "w3": w3,
                "bT3": bT3,
                "ident": eye,
                "UpT": np.ascontiguousarray(Up[ru].T).astype(bf16),
                "PuT": np.ascontiguousarray(Pu[rp].T).astype(bf16),
                "TarT": np.ascontiguousarray(Tar[re_].T).astype(bf16),
                "SrcT": np.ascontiguousarray(Src[rp].T).astype(bf16),
                "GeoT": np.ascontiguousarray(Geo[rp].T).astype(bf16),
            }
        )
    return in_maps


def _assemble(results, user_idx):
    f32 = np.float32
    hg = np.empty((P, D), f32)
    geo = np.empty((P, D), f32)
    tr = np.empty((P, D), f32)
    hgu = np.empty((U, D), f32)
    geou = np.empty((U, D), f32)
    for i in range(NCORES):
        rp = slice(PP * i, PP * (i + 1))
        ru = slice(UU * i, UU * (i + 1))
        pois = results[i]["poisT_o"]
        hg[rp] = pois[0].T
        geo[rp] = pois[1].T
        tr[rp] = pois[2].T
        users = results[i]["usersT_o"]
        hgu[ru] = users[:, :UU].T
        geou[ru] = users[:, UU:].T
    idx = np.asarray(user_idx)
    return np.concatenate([hg, geo, tr, hgu[idx], geou[idx]], axis=0)


def _run(inputs, trace=False, **spmd_kwargs):
    nc = _get_nc()
    in_maps = _shard_inputs(inputs)
    res = run_bass_kernel_spmd(
        nc, in_maps, list(range(NCORES)), trace=trace, **spmd_kwargs
    )
    return _assemble(res.results, inputs["user_idx"]), res


def kernel(**inputs):
    return _run(inputs)[0]


if __name__ == "__main__":
    import pickle

    with open("/tmp/inputs.pkl", "rb") as f:
        inputs = pickle.load(f)
    out = kernel(**inputs)
    exp = np.load("/tmp/expected.npy")
    rel = np.linalg.norm(out - exp) / np.linalg.norm(exp)
    print("Relative error:", rel)



# revision 6
# speedup vs baseline: 1.6550x; 1.6550x over previous
"""Trainium2 Bass kernel for nn_HODE_MDP (hypergraph ODE message passing).

Math (T_UP = T_GEO = T_P2P = 1.0, ALPHA = 0.8):
    pe  = poi_emb_weight[:-1]                      # [P, D]
    x/s/g = pe * sigmoid(pe @ W_t + b_t)           # col / seq / geo gates
    hg_pois    = x + HG_pu @ (HG_up @ x)
    geo_pois   = g + 0.4 * (poi_geo_graph @ g)
    trans_pois = s + HG_poi_src @ (HG_poi_tar @ s)
    hg_users   = (HG_up @ hg_pois)[user_idx]
    geo_users  = (HG_up @ geo_pois)[user_idx]
    out = concat([hg_pois, geo_pois, trans_pois, hg_users, geo_users])

Distribution (8 NeuronCores), v2 — contract-dim sharding upstream:
  * y_up = HG_up@x, y_tar = Tar@s, Geo@g are sharded over the CONTRACT
    dim (P): each core uses only its LOCAL gate block (gates computed
    for the own 1024-row block only) against the matching column block
    of each matrix, producing full-width partials.  One fused AllReduce
    ([D, U+E] bf16) combines y_up|y_tar; a ReduceScatter combines the
    geo partial straight into each core's own row-slice.
  * hg/trans deltas row-shard over P (full y_up/y_tar stationary after
    the AllReduce).
  * user embeddings: host pre-gathers Up[user_idx] -> [B, P]; each core
    contracts its local P-block of that against its local hg/geo pois
    blocks, and the HOST sums the 8 partial [D, 2B] outputs (no
    device collective for users at all).

All big streams are fp8 e4m3 with power-of-two pre-scaling (host side)
and DoubleRow matmuls (2 k-tiles per instruction = 2x PE throughput).
The delta terms are ~3e-3 of the residual scale, so fp8 on the delta
paths costs ~6e-5 relative error; the user matmul runs in bf16
(~4.5e-3 on the tiny user rows).  Measured end-to-end ~1e-4.
"""

import sys

if "/opt/trn_rl_repo" not in sys.path:
    sys.path.insert(0, "/opt/trn_rl_repo")

import numpy as np
import ml_dtypes

import concourse.bass as bass  # noqa: F401
import concourse.bacc as bacc
import concourse.mybir as mybir
import concourse.tile as tile
from concourse.bass_utils import run_bass_kernel_spmd

F32 = mybir.dt.float32
BF16 = mybir.dt.bfloat16
FP8 = mybir.dt.float8e4
SIG = mybir.ActivationFunctionType.Sigmoid
COPY = mybir.ActivationFunctionType.Copy
MULT = mybir.AluOpType.mult
ADD = mybir.AluOpType.add
BYPASS = mybir.AluOpType.bypass
DR = mybir.MatmulPerfMode.DoubleRow

NCORES = 8
P, U, E, D, B = 8192, 4096, 4096, 128, 1024
PP = P // NCORES            # 1024 rows per core
KL = PP // 128              # 8 local k-tiles
RG = [list(range(NCORES))]

SX = 128.0                  # gate -> fp8 scale (2^7)
SAR = 2.0 ** -7             # psum -> AllReduce payload scale
SHG = 2.0 ** -35            # C-phase psum -> f32 delta scale
GEO_SCALE = 0.4 * 2.0 ** -18  # geo payload (2^18 * Geo@g) -> delta

_CACHE: dict = {}


def _build_nc():
    nc = bacc.Bacc(
        "TRN2",
        target_bir_lowering=False,
        debug=False,
        enable_asserts=False,
        num_devices=NCORES,
    )

    # ---- per-core DRAM I/O ----------------------------------------------
    peT_bf = nc.dram_tensor("peT_bf", [D, PP], BF16, kind="ExternalInput").ap()
    peT_f = nc.dram_tensor("peT_f", [D, PP], F32, kind="ExternalInput").ap()
    w3 = nc.dram_tensor("w3", [D, 3, D], BF16, kind="ExternalInput").ap()
    bT3 = nc.dram_tensor("bT3", [D, 3], F32, kind="ExternalInput").ap()
    ident_f = nc.dram_tensor("ident_f", [D, D], F32, kind="ExternalInput").ap()
    ident_b = nc.dram_tensor("ident_b", [D, D], BF16, kind="ExternalInput").ap()
    UpT = nc.dram_tensor("UpT", [PP, U], FP8, kind="ExternalInput").ap()
    TarT = nc.dram_tensor("TarT", [PP, E], FP8, kind="ExternalInput").ap()
    GeoT = nc.dram_tensor("GeoT", [PP, P], FP8, kind="ExternalInput").ap()
    PuT = nc.dram_tensor("PuT", [U, PP], FP8, kind="ExternalInput").ap()
    SrcT = nc.dram_tensor("SrcT", [E, PP], FP8, kind="ExternalInput").ap()
    SelT = nc.dram_tensor("SelT", [PP, B], BF16, kind="ExternalInput").ap()

    poisT_o = nc.dram_tensor("poisT_o", [3, D, PP], F32, kind="ExternalOutput").ap()
    usersT_o = nc.dram_tensor("usersT_o", [D, 2 * B], F32, kind="ExternalOutput").ap()

    with tile.TileContext(nc) as tc:
        with (
            tc.tile_pool(name="const", bufs=1) as constp,
            tc.tile_pool(name="mat", bufs=5) as matp,
            tc.tile_pool(name="big32", bufs=6) as big32,
            tc.tile_pool(name="stage", bufs=3) as stagep,
            tc.tile_pool(name="outp", bufs=1) as outp,
            tc.tile_pool(name="psg", bufs=2, space="PSUM") as psg,
            tc.tile_pool(name="pstr", bufs=2, space="PSUM") as pstr,
            tc.tile_pool(name="dram", bufs=1, space="DRAM") as dramp,
        ):
            # ---- collective DRAM buffers --------------------------------
            cc_ar_in = dramp.tile([D, U + E], BF16, name="cc_ar_in")
            cc_ar_out = dramp.tile(
                [D, U + E], BF16, addr_space="Shared", name="cc_ar_out"
            )
            cc_rs_in = dramp.tile([NCORES * D, PP], BF16, name="cc_rs_in")
            cc_rs_out = dramp.tile([D, PP], BF16, name="cc_rs_out")

            # ---- constants ----------------------------------------------
            sb_w = constp.tile([D, 3, D], BF16, name="sb_w")
            nc.gpsimd.dma_start(sb_w[:], w3)
            sb_bT = constp.tile([D, 3], F32, name="sb_bT")
            nc.gpsimd.dma_start(sb_bT[:], bT3)
            sb_idf = constp.tile([D, D], F32, name="sb_idf")
            nc.gpsimd.dma_start(sb_idf[:], ident_f)
            sb_idb = constp.tile([D, D], BF16, name="sb_idb")
            nc.gpsimd.dma_start(sb_idb[:], ident_b)
            sb_peb = constp.tile([D, PP], BF16, name="sb_peb")
            nc.gpsimd.dma_start(sb_peb[:], peT_bf)
            sb_pef = constp.tile([D, PP], F32, name="sb_pef")
            nc.gpsimd.dma_start(sb_pef[:], peT_f)

            # gate natural fp8 tiles (stationary lhs for streams)
            nat = [
                constp.tile([128, KL, 128], FP8, name=f"nat{t}") for t in range(3)
            ]
            # own-block gates, transposed f32 (residual path)
            ownT = [
                big32.tile([D, PP], F32, tag="big32", name=f"ownT{t}")
                for t in range(3)
            ]

            # ---- gates (own block only) ---------------------------------
            for t in range(3):
                psz = psg.tile([D, PP], F32, tag="psg")
                for h in range(2):
                    cols = slice(512 * h, 512 * (h + 1))
                    nc.tensor.matmul(
                        psz[:, cols], sb_w[:, t, :], sb_peb[:, cols],
                        start=True, stop=True,
                    )
                sigT = stagep.tile([D, PP], F32, tag="sig")
                for h in range(2):
                    cols = slice(512 * h, 512 * (h + 1))
                    nc.scalar.activation(
                        sigT[:, cols], psz[:, cols], SIG, bias=sb_bT[:, t : t + 1]
                    )
                nc.vector.tensor_mul(ownT[t][:], sb_pef[:], sigT[:])
                for j in range(2):
                    pst = pstr.tile([128, 512], F32, tag="pstr")
                    for m in range(4):
                        c = (4 * j + m) * 128
                        nc.tensor.transpose(
                            pst[:, m * 128 : (m + 1) * 128],
                            ownT[t][:, c : c + 128],
                            sb_idf[:],
                        )
                    nc.scalar.activation(
                        nat[t][:, 4 * j : 4 * j + 4, :], pst[:], COPY, scale=SX
                    )

            # ---- stream helper: contract-sharded partial ----------------
            def partial_stream(matT, n_out, lhs, pay, pay_off, n_tiles, eng_list):
                """pay[:, pay_off + n*1024 ...] = SAR * sum_k lhs_k.T @ matT.

                matT: DRAM [PP, n_out] fp8; loaded as n_tiles chunks of
                [128, 2*KT, n_out/...]; DoubleRow pairs over the 8 local
                k-tiles; psum out-chunks of 1024 cols.
                """
                kt_per = KL // n_tiles  # k-tiles per DMA chunk
                chunks = []
                for a in range(n_tiles):
                    ch = matp.tile([128, kt_per, n_out], FP8, tag="mat")
                    eng = nc.sync if a % 2 == 0 else nc.scalar
                    eng.dma_start(
                        ch[:],
                        matT[a * kt_per * 128 : (a + 1) * kt_per * 128, :].rearrange(
                            "(a p) n -> p a n", p=128
                        ),
                    )
                    chunks.append(ch)
                n_po = n_out // 1024
                for n in range(n_po):
                    ps = psg.tile([D, 1024], F32, tag="psg")
                    for j in range(KL // 2):  # DR pairs
                        a, jj = (2 * j) // kt_per, (2 * j) % kt_per
                        rhs3 = chunks[a]
                        for h in range(2):
                            cols = slice(n * 1024 + h * 512, n * 1024 + h * 512 + 512)
                            nc.tensor.matmul(
                                ps[:, h * 512 : h * 512 + 512],
                                lhs[:, 2 * j : 2 * j + 2, :],
                                rhs3[:, jj : jj + 2, cols],
                                start=(j == 0),
                                stop=(j == KL // 2 - 1),
                                perf_mode=DR,
                            )
                    eng = eng_list[n % len(eng_list)]
                    if eng is nc.vector:
                        eng.tensor_scalar_mul(
                            pay[:, pay_off + n * 1024 : pay_off + (n + 1) * 1024],
                            ps[:],
                            SAR,
                        )
                    else:
                        eng.activation(
                            pay[:, pay_off + n * 1024 : pay_off + (n + 1) * 1024],
                            ps[:],
                            COPY,
                            scale=SAR,
                        )

            # ---- B1/B2: y_up | y_tar partials + fused AllReduce ---------
            yuyt_pay = constp.tile([D, U + E], BF16, name="yuyt_pay")
            partial_stream(UpT, U, nat[0], yuyt_pay, 0, 2, [nc.vector])
            for n in range(4):
                nc.gpsimd.dma_start(
                    cc_ar_in[:, n * 1024 : (n + 1) * 1024],
                    yuyt_pay[:, n * 1024 : (n + 1) * 1024],
                )
            partial_stream(TarT, E, nat[1], yuyt_pay, U, 2, [nc.vector])
            for n in range(4):
                nc.gpsimd.dma_start(
                    cc_ar_in[:, U + n * 1024 : U + (n + 1) * 1024],
                    yuyt_pay[:, U + n * 1024 : U + (n + 1) * 1024],
                )
            nc.gpsimd.collective_compute(
                "AllReduce",
                ADD,
                replica_groups=RG,
                ins=[cc_ar_in[:].opt()],
                outs=[cc_ar_out[:].opt()],
            )

            # ---- B3: geo partial + ReduceScatter ------------------------
            geo_part = constp.tile([D, P], BF16, name="geo_part")
            partial_stream(GeoT, P, nat[2], geo_part, 0, 4, [nc.scalar])
            nc.gpsimd.dma_start(
                cc_rs_in[:].rearrange("(r p) c -> p r c", p=128),
                geo_part[:].rearrange("p (r c) -> p r c", r=NCORES),
            )
            nc.gpsimd.collective_compute(
                "ReduceScatter",
                ADD,
                replica_groups=RG,
                ins=[cc_rs_in[:].opt()],
                outs=[cc_rs_out[:].opt()],
            )

            # ---- AllReduce readback + natural fp8 tiles -----------------
            yuyt_full = constp.tile([D, U + E], BF16, name="yuyt_full")
            nc.gpsimd.dma_start(yuyt_full[:, :U], cc_ar_out[:, :U])
            nc.gpsimd.dma_start(yuyt_full[:, U:], cc_ar_out[:, U:])
            yu_nat = constp.tile([128, U // 128, 128], FP8, name="yu_nat")
            yt_nat = constp.tile([128, E // 128, 128], FP8, name="yt_nat")
            for dst, off in ((yu_nat, 0), (yt_nat, U)):
                for j in range(8):
                    pst = pstr.tile([128, 512], BF16, tag="pstr")
                    for m in range(4):
                        c = off + (4 * j + m) * 128
                        nc.tensor.transpose(
                            pst[:, m * 128 : (m + 1) * 128],
                            yuyt_full[:, c : c + 128],
                            sb_idb[:],
                        )
                    nc.vector.tensor_copy(dst[:, 4 * j : 4 * j + 4, :], pst[:])

            # ---- row-sharded delta stream (C1 / C2) ---------------------
            def delta_stream(matT, n_k, lhs):
                """psum [D, PP] = sum_k lhs_k.T @ matT[k, :] (fp8 DR)."""
                kt_per = n_k // 2
                chunks = []
                for a in range(2):
                    ch = matp.tile([128, kt_per, PP], FP8, tag="mat")
                    eng = nc.sync if a % 2 == 0 else nc.scalar
                    eng.dma_start(
                        ch[:],
                        matT[a * kt_per * 128 : (a + 1) * kt_per * 128, :].rearrange(
                            "(a p) n -> p a n", p=128
                        ),
                    )
                    chunks.append(ch)
                ps = psg.tile([D, PP], F32, tag="psg")
                for j in range(n_k // 2):
                    a, jj = (2 * j) // kt_per, (2 * j) % kt_per
                    for h in range(2):
                        nc.tensor.matmul(
                            ps[:, h * 512 : h * 512 + 512],
                            lhs[:, 2 * j : 2 * j + 2, :],
                            chunks[a][:, jj : jj + 2, h * 512 : h * 512 + 512],
                            start=(j == 0),
                            stop=(j == n_k // 2 - 1),
                            perf_mode=DR,
                        )
                return ps

            def make_nat(srcT, dst):
                """PE-transpose f32 [D, PP] -> natural bf16 [128, KL, 128]."""
                for j in range(2):
                    pst = pstr.tile([128, 512], F32, tag="pstr")
                    for m in range(4):
                        c = (4 * j + m) * 128
                        nc.tensor.transpose(
                            pst[:, m * 128 : (m + 1) * 128],
                            srcT[:, c : c + 128],
                            sb_idf[:],
                        )
                    nc.vector.tensor_copy(dst[:, 4 * j : 4 * j + 4, :], pst[:])

            # C1: hg_pois = x + Pu @ y_up
            ps_hg = delta_stream(PuT, U // 128, yu_nat)
            hg_poisT = big32.tile([D, PP], F32, tag="big32", name="hg_poisT")
            nc.vector.scalar_tensor_tensor(
                hg_poisT[:], ps_hg[:], SHG, ownT[0][:], MULT, ADD
            )
            nc.gpsimd.dma_start(poisT_o[0], hg_poisT[:])
            hg_nat = constp.tile([128, KL, 128], BF16, name="hg_nat")
            make_nat(hg_poisT, hg_nat)

            # C2: trans_pois = s + Src @ y_tar
            ps_tr = delta_stream(SrcT, E // 128, yt_nat)
            trans_poisT = big32.tile([D, PP], F32, tag="big32", name="trans_poisT")
            nc.vector.scalar_tensor_tensor(
                trans_poisT[:], ps_tr[:], SHG, ownT[1][:], MULT, ADD
            )
            nc.gpsimd.dma_start(poisT_o[2], trans_poisT[:])

            # ---- geo finalize (ReduceScatter output) --------------------
            geo_sum = stagep.tile([D, PP], BF16, tag="sig", name="geo_sum")
            nc.gpsimd.dma_start(geo_sum[:], cc_rs_out[:])
            geo_poisT = big32.tile([D, PP], F32, tag="big32", name="geo_poisT")
            nc.vector.scalar_tensor_tensor(
                geo_poisT[:], geo_sum[:], GEO_SCALE, ownT[2][:], MULT, ADD
            )
            nc.gpsimd.dma_start(poisT_o[1], geo_poisT[:])
            geo_nat = constp.tile([128, KL, 128], BF16, name="geo_nat")
            make_nat(geo_poisT, geo_nat)

            # ---- D: user partials (bf16, host reduces) ------------------
            sel = matp.tile([128, KL, B], BF16, tag="mat")
            nc.sync.dma_start(
                sel[:], SelT[:].rearrange("(a p) n -> p a n", p=128)
            )
            ps_hu = psg.tile([D, B], F32, tag="psg")
            ps_gu = psg.tile([D, B], F32, tag="psg")
            for k in range(KL):
                for h in range(2):
                    cols = slice(h * 512, h * 512 + 512)
                    nc.tensor.matmul(
                        ps_hu[:, cols], hg_nat[:, k, :], sel[:, k, cols],
                        start=(k == 0), stop=(k == KL - 1),
                    )
                    nc.tensor.matmul(
                        ps_gu[:, cols], geo_nat[:, k, :], sel[:, k, cols],
                        start=(k == 0), stop=(k == KL - 1),
                    )
            users_sb = outp.tile([D, 2 * B], F32, name="users_sb")
            nc.vector.tensor_copy(users_sb[:, :B], ps_hu[:])
            nc.vector.tensor_copy(users_sb[:, B:], ps_gu[:])
            nc.gpsimd.dma_start(usersT_o, users_sb[:])

    nc.compile()
    return nc


def _get_nc():
    if "nc" not in _CACHE:
        _CACHE["nc"] = _build_nc()
    return _CACHE["nc"]


def _shard_inputs(inputs):
    f32 = np.float32
    bf16 = ml_dtypes.bfloat16
    fp8 = ml_dtypes.float8_e4m3
    pe = np.asarray(inputs["poi_emb_weight"], f32)[:P]
    peT = np.ascontiguousarray(pe.T)                     # [D, P]
    w3 = np.stack(
        [
            np.asarray(inputs["w_gate_col"], f32),
            np.asarray(inputs["w_gate_seq"], f32),
            np.asarray(inputs["w_gate_geo"], f32),
        ],
        axis=1,
    ).astype(bf16)                                        # [D, 3, D]
    bT3 = np.stack(
        [
            np.asarray(inputs["b_gate_col"], f32)[0],
            np.asarray(inputs["b_gate_seq"], f32)[0],
            np.asarray(inputs["b_gate_geo"], f32)[0],
        ],
        axis=1,
    )
    eye = np.eye(D, dtype=f32)
    idx = np.asarray(inputs["user_idx"]).astype(np.int64)
    Up = np.asarray(inputs["HG_up"], f32)                 # [U, P]
    Pu = np.asarray(inputs["HG_pu"], f32)                 # [P, U]
    Tar = np.asarray(inputs["HG_poi_tar"], f32)           # [E, P]
    Src = np.asarray(inputs["HG_poi_src"], f32)           # [P, E]
    Geo = np.asarray(inputs["poi_geo_graph"], f32)        # [P, P]
    UpT_full = Up.T                                       # [P, U] view
    TarT_full = Tar.T                                     # [P, E] view
    GeoT_full = Geo.T                                     # [P, P] view
    S18, S17 = 2.0 ** 18, 2.0 ** 17

    in_maps = []
    for i in range(NCORES):
        rp = slice(PP * i, PP * (i + 1))
        in_maps.append(
            {
                "peT_bf": peT[:, rp].astype(bf16),
                "peT_f": np.ascontiguousarray(peT[:, rp]),
                "w3": w3,
                "bT3": bT3,
                "ident_f": eye,
                "ident_b": eye.astype(bf16),
                "UpT": (UpT_full[rp] * S18).astype(fp8),
                "TarT": (TarT_full[rp] * S18).astype(fp8),
                "GeoT": (GeoT_full[rp] * S18).astype(fp8),
                "PuT": (Pu[rp].T * S17).astype(fp8),
                "SrcT": (Src[rp].T * S17).astype(fp8),
                "SelT": np.ascontiguousarray(Up[idx, rp.start : rp.stop].T).astype(
                    bf16
                ),
            }
        )
    return in_maps


def _assemble(results):
    f32 = np.float32
    hg = np.empty((P, D), f32)
    geo = np.empty((P, D), f32)
    tr = np.empty((P, D), f32)
    users = np.zeros((D, 2 * B), f32)
    for i in range(NCORES):
        rp = slice(PP * i, PP * (i + 1))
        pois = results[i]["poisT_o"]
        hg[rp] = pois[0].T
        geo[rp] = pois[1].T
        tr[rp] = pois[2].T
        users += results[i]["usersT_o"]
    return np.concatenate([hg, geo, tr, users[:, :B].T, users[:, B:].T], axis=0)


def _run(inputs, trace=False, **spmd_kwargs):
    nc = _get_nc()
    in_maps = _shard_inputs(inputs)
    res = run_bass_kernel_spmd(
        nc, in_maps, list(range(NCORES)), trace=trace, **spmd_kwargs
    )
    return _assemble(res.results), res


def kernel(**inputs):
    return _run(inputs)[0]


if __name__ == "__main__":
    import pickle

    with open("/tmp/inputs.pkl", "rb") as f:
        inputs = pickle.load(f)
    out = kernel(**inputs)
    exp = np.load("/tmp/expected.npy")
    rel = np.linalg.norm(out - exp) / np.linalg.norm(exp)
    print("Relative error:", rel)
